# revision 15
# baseline (speedup 1.0000x reference)
"""Trainium2 Bass kernel for nn_LinearAttention_40544491274679.

Computation: token embedding gather -> L=2 layers of
  [3x causal-conv FFN ladders (F->I, I->I k=3, I->F), feature-dim cumsum,
   position-normalized cell + momentum coupling] ->
1x1 conv to logits -> log_softmax -> mean NLL (scalar).

Sharding: data-parallel over (batch, sequence-quarter) = 8 shards, one per
NeuronCore. Each core processes 512 output positions with a 4-column left
halo (2 causal-conv lookbacks of 2 positions each across the 2 layers).
Weights are replicated (streamed per-FFN from HBM, bf16).

Layout on device: channels on partitions (chunks of 128), positions on the
free dim (W=516 columns: 4 halo + 512). All convs are matmuls with the
contraction dim on partitions; the feature-dim cumsum is a triangular-ones
matmul; channel-dim reductions (mean/L2 norm, logsumexp, target-gather) are
ones-vector matmuls; per-position row stats are broadcast back across
partitions with K=1 matmuls.
"""

import math
from contextlib import ExitStack

import numpy as np
import ml_dtypes

import concourse.bass as bass
import concourse.tile as tile
from concourse import bacc, mybir
from concourse import bass_utils

# Problem constants (hardcoded; kernel.py must be self-contained).
B, S, F, I, KW, L, C = 2, 2048, 512, 1024, 3, 2, 256
BETA = 0.99
INIT_SCALE = L ** -0.5
NCORES = 8
CHUNK = 512          # output positions per core
HALO = 4             # left halo columns
W = CHUNK + HALO     # working width per core
PAD_IDX = 300.0      # sentinel index -> all-zero one-hot -> x = 0 (causal pad)

dt = mybir.dt
AF = mybir.ActivationFunctionType
OP = mybir.AluOpType

TRACE = False  # set True from test harness to capture an NTFF profile

_CACHE = {}


def _halves(lo):
    """Split columns [lo, W) into two matmul-sized (<=512) halves."""
    total = W - lo
    h1 = total // 2
    return [(lo, h1), (lo + h1, total - h1)]


def _bcast_ap(dram_handle, n):
    """AP that reads a 1-D DRAM row replicated across 128 partitions."""
    a = dram_handle[:]
    return bass.AP(tensor=a.tensor, offset=a.offset,
                   ap=[[0, 128]] + [list(x) for x in a.ap])


def _build():
    nc = bacc.Bacc("TRN2", target_bir_lowering=False, debug=False,
                   num_devices=NCORES)

    # ---- DRAM I/O ----
    d_emb = nc.dram_tensor("emb32", [128, 2 * 1024], dt.float32, kind="ExternalInput")
    d_ow = nc.dram_tensor("owT", [128, 8 * 256], dt.bfloat16, kind="ExternalInput")
    d_ob = nc.dram_tensor("ob2", [128, 2], dt.float32, kind="ExternalInput")
    d_fe = nc.dram_tensor("fe4", [128, 4], dt.float32, kind="ExternalInput")
    d_tri = nc.dram_tensor("triU", [128, 128], dt.bfloat16, kind="ExternalInput")
    d_w0, d_w1, d_w2 = {}, {}, {}
    for l in range(L):
        for j in range(3):
            d_w0[(l, j)] = nc.dram_tensor(f"w0_{l}{j}", [128, 4 * 1024], dt.bfloat16,
                                          kind="ExternalInput")
            d_w1[(l, j)] = nc.dram_tensor(f"w1_{l}{j}", [128, 24 * 1024], dt.bfloat16,
                                          kind="ExternalInput")
            d_w2[(l, j)] = nc.dram_tensor(f"w2_{l}{j}", [128, 8 * 512], dt.bfloat16,
                                          kind="ExternalInput")
    d_inp = nc.dram_tensor("inprow", [W], dt.float32, kind="ExternalInput")
    d_tgt = nc.dram_tensor("tgtrow", [CHUNK], dt.float32, kind="ExternalInput")
    d_idv = nc.dram_tensor("idvrow", [W], dt.float32, kind="ExternalInput")
    d_m4 = nc.dram_tensor("m4row", [HALO], dt.float32, kind="ExternalInput")
    d_nll = nc.dram_tensor("nll", [1, CHUNK], dt.float32, kind="ExternalOutput")

    with tile.TileContext(nc) as tc, ExitStack() as ctx:
        sb = ctx.enter_context(tc.tile_pool(name="sb", bufs=1))
        ps = ctx.enter_context(tc.tile_pool(name="ps", bufs=1,
                                            space=bass.MemorySpace.PSUM))

        def pc_tile(w):
            return ps.tile([128, w], dt.float32, tag="pc", bufs=5, name="pc")

        def pb_tile(w):
            return ps.tile([128, w], dt.float32, tag="pb", bufs=2, name="pb")

        def pr_tile(w):
            return ps.tile([1, w], dt.float32, tag="pr", bufs=1, name="pr")

        # ---- constants / broadcast inputs ----
        emb_sb = sb.tile([128, 2048], dt.float32, tag="emb", name="emb_sb")
        nc.sync.dma_start(out=emb_sb[:], in_=d_emb[:])
        ow_sb = sb.tile([128, 2048], dt.bfloat16, tag="ow", name="ow_sb")
        nc.sync.dma_start(out=ow_sb[:], in_=d_ow[:])
        ob_sb = sb.tile([128, 2], dt.float32, tag="ob", name="ob_sb")
        nc.sync.dma_start(out=ob_sb[:], in_=d_ob[:])
        fe_sb = sb.tile([128, 4], dt.float32, tag="fe", name="fe_sb")
        nc.sync.dma_start(out=fe_sb[:], in_=d_fe[:])
        tri_sb = sb.tile([128, 128], dt.bfloat16, tag="tri", name="tri_sb")
        nc.sync.dma_start(out=tri_sb[:], in_=d_tri[:])

        idb = sb.tile([128, W], dt.float32, tag="idb", name="idb")
        nc.sync.dma_start(out=idb[:], in_=_bcast_ap(d_idv, W))
        m4b = sb.tile([128, HALO], dt.float32, tag="m4b", name="m4b")
        nc.sync.dma_start(out=m4b[:], in_=_bcast_ap(d_m4, HALO))
        m4bh = sb.tile([128, HALO], dt.bfloat16, tag="m4bh", name="m4bh")
        nc.vector.tensor_copy(m4bh[:], m4b[:])

        ones_blk = sb.tile([128, 128], dt.bfloat16, tag="oblk", name="ones_blk")
        nc.vector.memset(ones_blk[:], 1.0)
        ones_cf = sb.tile([128, 1], dt.float32, tag="ocf", name="ones_cf")
        nc.vector.memset(ones_cf[:], 1.0)
        ones_cb = sb.tile([128, 1], dt.bfloat16, tag="ocb", name="ones_cb")
        nc.vector.memset(ones_cb[:], 1.0)
        bcmean = sb.tile([1, 128], dt.float32, tag="bcm", name="bcmean")
        nc.vector.memset(bcmean[:], 1.0 / F)
        g_scale = float((np.float32(1.0) - np.float32(BETA)) * np.float32(INIT_SCALE))
        bcg = sb.tile([1, 128], dt.float32, tag="bcg", name="bcg")
        nc.vector.memset(bcg[:], g_scale)

        iota_i = sb.tile([128, 1], dt.int32, tag="ioi", name="iota_i")
        nc.gpsimd.iota(iota_i[:], [[0, 1]], base=0, channel_multiplier=1)
        iota_f = []
        for ck in range(2):
            t = sb.tile([128, 1], dt.float32, tag=f"iof{ck}", name=f"iota_f{ck}")
            if ck == 0:
                nc.vector.tensor_copy(t[:], iota_i[:])
            else:
                nc.vector.tensor_scalar_add(t[:], iota_f[0][:], 128.0)
            iota_f.append(t)

        # ---- embedding gather via one-hot fp32 matmul ----
        inb = sb.tile([128, W], dt.float32, tag="ibc", name="inb")
        nc.sync.dma_start(out=inb[:], in_=_bcast_ap(d_inp, W))
        oh = sb.tile([128, 2, W], dt.float32, tag="oh", name="oh")
        for ck in range(2):
            nc.vector.tensor_scalar(oh[:, ck, :], inb[:], iota_f[ck][:], None,
                                    op0=OP.is_equal)
        a_t = sb.tile([128, 4, W], dt.float32, tag="sa", name="a_t")
        b_t = sb.tile([128, 4, W], dt.float32, tag="sb_", name="b_t")
        for fi in range(8):
            dst = a_t if fi < 4 else b_t
            for (lo, w) in _halves(0):
                pt = pc_tile(w)
                for ck in range(2):
                    nc.tensor.matmul(pt[:, :w],
                                     emb_sb[:, ck * 1024 + fi * 128:
                                            ck * 1024 + (fi + 1) * 128],
                                     oh[:, ck, lo:lo + w],
                                     start=(ck == 0), stop=(ck == 1))
                nc.vector.tensor_copy(dst[:, fi % 4, lo:lo + w], pt[:, :w])

        # ---- layers ----
        c_tiles = [sb.tile([128, 4, W], dt.float32, tag=f"scc{l}", name=f"c_t{l}")
                   for l in range(L)]
        for l in range(L):
            r0, r1 = 2 * l, 2 * l + 2
            # h = mask * (b + fe), bf16
            h_bf = sb.tile([128, 4, W], dt.bfloat16, tag="h", bufs=2, name="h_bf")
            for fk in range(4):
                nc.vector.tensor_scalar(h_bf[:, fk, r0:], b_t[:, fk, r0:],
                                        fe_sb[:, fk:fk + 1], None, op0=OP.add)
                nc.vector.tensor_tensor(h_bf[:, fk, r0:HALO], h_bf[:, fk, r0:HALO],
                                        m4bh[:, r0:HALO], op=OP.mult)

            y_t = sb.tile([128, 4, W], dt.float32, tag="y", name="y_t")
            sc_t = sb.tile([128, 4, W], dt.float32, tag="sct", name="sc_t")
            dbf = sb.tile([128, 4, W], dt.bfloat16, tag="dbf", name="dbf")

            for j in range(3):
                w0t = sb.tile([128, 4096], dt.bfloat16, tag="w0", bufs=1, name="w0t")
                nc.sync.dma_start(out=w0t[:], in_=d_w0[(l, j)][:])
                w1t = sb.tile([128, 24576], dt.bfloat16, tag="w1", bufs=1, name="w1t")
                for q in range(4):
                    nc.sync.dma_start(out=w1t[:, q * 6144:(q + 1) * 6144],
                                      in_=d_w1[(l, j)][:, q * 6144:(q + 1) * 6144])
                w2t = sb.tile([128, 4096], dt.bfloat16, tag="w2", bufs=2, name="w2t")
                nc.sync.dma_start(out=w2t[:], in_=d_w2[(l, j)][:])

                # conv0 (1x1, F->I) + relu
                x1 = sb.tile([128, 8, W], dt.bfloat16, tag="x1", name="x1")
                for (lo, w) in _halves(r0):
                    for ic in range(8):
                        pt = pc_tile(w)
                        for fk in range(4):
                            nc.tensor.matmul(pt[:, :w],
                                             w0t[:, fk * 1024 + ic * 128:
                                                 fk * 1024 + (ic + 1) * 128],
                                             h_bf[:, fk, lo:lo + w],
                                             start=(fk == 0), stop=(fk == 3))
                        nc.scalar.activation(x1[:, ic, lo:lo + w], pt[:, :w], AF.Relu)

                # conv1 (k=3 causal, I->I) + relu
                x2 = sb.tile([128, 8, W], dt.bfloat16, tag="x2", name="x2")
                for oi in range(8):
                    for (lo, w) in _halves(r1):
                        pt = pc_tile(w)
                        first = True
                        for k in range(KW):
                            for ik in range(8):
                                nc.tensor.matmul(
                                    pt[:, :w],
                                    w1t[:, oi * 3072 + (k * 8 + ik) * 128:
                                        oi * 3072 + (k * 8 + ik + 1) * 128],
                                    x1[:, ik, lo - 2 + k:lo - 2 + k + w],
                                    start=first, stop=(k == KW - 1 and ik == 7))
                                first = False
                        nc.scalar.activation(x2[:, oi, lo:lo + w], pt[:, :w], AF.Relu)

                # conv2 (1x1, I->F); evacuation depends on which FFN this is
                for (lo, w) in _halves(r1):
                    for fc in range(4):
                        pt = pc_tile(w)
                        for ik in range(8):
                            nc.tensor.matmul(pt[:, :w],
                                             w2t[:, ik * 512 + fc * 128:
                                                 ik * 512 + (fc + 1) * 128],
                                             x2[:, ik, lo:lo + w],
                                             start=(ik == 0), stop=(ik == 7))
                        if j == 0:
                            # d -> bf16 (feeds the cumsum matmul)
                            nc.vector.tensor_copy(dbf[:, fc, lo:lo + w], pt[:, :w])
                        elif j == 1:
                            nc.vector.tensor_copy(sc_t[:, fc, lo:lo + w], pt[:, :w])
                        else:
                            # y += sh
                            nc.vector.tensor_tensor(y_t[:, fc, lo:lo + w],
                                                    y_t[:, fc, lo:lo + w],
                                                    pt[:, :w], op=OP.add)

                if j == 0:
                    # cumsum over features (triangular matmul), y = cum * inv_div
                    for (lo, w) in _halves(r1):
                        for fm in range(4):
                            pt = pc_tile(w)
                            for fk in range(fm + 1):
                                lhs = ones_blk if fk < fm else tri_sb
                                nc.tensor.matmul(pt[:, :w], lhs[:],
                                                 dbf[:, fk, lo:lo + w],
                                                 start=(fk == 0), stop=(fk == fm))
                            nc.vector.tensor_tensor(y_t[:, fm, lo:lo + w],
                                                    pt[:, :w], idb[:, lo:lo + w],
                                                    op=OP.mult)
                if j == 1:
                    # y = (cum * inv_div) * sc  (single 3-D op)
                    nc.vector.tensor_tensor(y_t[:, :, r1:], y_t[:, :, r1:],
                                            sc_t[:, :, r1:], op=OP.mult)

            # ---- norm stats ----
            ysq = sb.tile([128, 4, W], dt.bfloat16, tag="dbf", name="ysq")
            nc.vector.tensor_mul(ysq[:, :, r1:], y_t[:, :, r1:], y_t[:, :, r1:])
            srow = sb.tile([1, W], dt.float32, tag="srow", bufs=2, name="srow")
            qrow = sb.tile([1, W], dt.float32, tag="qrow", bufs=2, name="qrow")
            for (lo, w) in _halves(r1):
                pt = pr_tile(w)
                for fk in range(4):
                    nc.tensor.matmul(pt[:1, :w], ones_cf[:], y_t[:, fk, lo:lo + w],
                                     start=(fk == 0), stop=(fk == 3))
                nc.vector.tensor_copy(srow[:, lo:lo + w], pt[:1, :w])
                pt2 = pr_tile(w)
                for fk in range(4):
                    nc.tensor.matmul(pt2[:1, :w], ones_cb[:], ysq[:, fk, lo:lo + w],
                                     start=(fk == 0), stop=(fk == 3))
                nc.vector.tensor_copy(qrow[:, lo:lo + w], pt2[:1, :w])
            # rows: nsq = max(q - s^2/F, 0); den = sqrt(nsq/F) + eps; g = 1/den
            rt = sb.tile([1, W], dt.float32, tag="rt", bufs=2, name="rt")
            nc.vector.tensor_mul(rt[:, r1:], srow[:, r1:], srow[:, r1:])
            nc.vector.scalar_tensor_tensor(rt[:, r1:], rt[:, r1:], -1.0 / F,
                                           qrow[:, r1:], op0=OP.mult, op1=OP.add)
            nc.vector.tensor_scalar_max(rt[:, r1:], rt[:, r1:], 0.0)
            rt2 = sb.tile([1, W], dt.float32, tag="rt2", bufs=2, name="rt2")
            nc.scalar.activation(rt2[:, r1:], rt[:, r1:], AF.Sqrt, scale=1.0 / F)
            nc.vector.tensor_scalar_add(rt2[:, r1:], rt2[:, r1:], 1e-5)
            grow = sb.tile([1, W], dt.float32, tag="grow", bufs=2, name="grow")
            nc.vector.reciprocal(grow[:, r1:], rt2[:, r1:])

            # ---- momentum coupling ----
            c_t = c_tiles[l]
            for (lo, w) in _halves(r1):
                pm = pb_tile(w)
                nc.tensor.matmul(pm[:, :w], bcmean[:], srow[:1, lo:lo + w],
                                 start=True, stop=True)
                pg = pb_tile(w)
                nc.tensor.matmul(pg[:, :w], bcg[:], grow[:1, lo:lo + w],
                                 start=True, stop=True)
                for fk in range(4):
                    nc.vector.tensor_tensor(y_t[:, fk, lo:lo + w],
                                            y_t[:, fk, lo:lo + w], pm[:, :w],
                                            op=OP.subtract)
                    nc.vector.tensor_tensor(y_t[:, fk, lo:lo + w],
                                            y_t[:, fk, lo:lo + w], pg[:, :w],
                                            op=OP.mult)
                    nc.vector.scalar_tensor_tensor(c_t[:, fk, lo:lo + w],
                                                   a_t[:, fk, lo:lo + w],
                                                   float(np.float32(BETA)),
                                                   y_t[:, fk, lo:lo + w],
                                                   op0=OP.mult, op1=OP.add)
                    nc.vector.tensor_tensor(b_t[:, fk, lo:lo + w],
                                            b_t[:, fk, lo:lo + w],
                                            c_t[:, fk, lo:lo + w], op=OP.add)
            a_t = c_t

        # ---- final: logits, log_softmax, NLL ----
        abf = sb.tile([128, 4, W], dt.bfloat16, tag="h", bufs=2, name="abf")
        bbf = sb.tile([128, 4, W], dt.bfloat16, tag="h", bufs=2, name="bbf")
        nc.vector.tensor_copy(abf[:, :, HALO:], a_t[:, :, HALO:])
        nc.vector.tensor_copy(bbf[:, :, HALO:], b_t[:, :, HALO:])
        logits = sb.tile([128, 2, W], dt.float32, tag="logits", name="logits")
        for cc in range(2):
            for (lo, w) in _halves(HALO):
                pt = pc_tile(w)
                for f2k in range(8):
                    src = abf if f2k < 4 else bbf
                    nc.tensor.matmul(pt[:, :w],
                                     ow_sb[:, f2k * 256 + cc * 128:
                                           f2k * 256 + (cc + 1) * 128],
                                     src[:, f2k % 4, lo:lo + w],
                                     start=(f2k == 0), stop=(f2k == 7))
                nc.scalar.activation(logits[:, cc, lo:lo + w], pt[:, :w],
                                     AF.Identity, bias=ob_sb[:, cc:cc + 1])

        # target one-hot + gather logit_tgt
        tgb = sb.tile([128, CHUNK], dt.float32, tag="ibc", name="tgb")
        nc.sync.dma_start(out=tgb[:], in_=_bcast_ap(d_tgt, CHUNK))
        oht = sb.tile([128, 2, W], dt.float32, tag="oh", name="oht")
        for cc in range(2):
            nc.vector.tensor_scalar(oht[:, cc, HALO:], tgb[:], iota_f[cc][:], None,
                                    op0=OP.is_equal)
        nc.vector.tensor_tensor(oht[:, :, HALO:], oht[:, :, HALO:],
                                logits[:, :, HALO:], op=OP.mult)
        ltrow = sb.tile([1, W], dt.float32, tag="ltrow", name="ltrow")
        for (lo, w) in _halves(HALO):
            pt = pr_tile(w)
            for cc in range(2):
                nc.tensor.matmul(pt[:1, :w], ones_cf[:], oht[:, cc, lo:lo + w],
                                 start=(cc == 0), stop=(cc == 1))
            nc.vector.tensor_copy(ltrow[:, lo:lo + w], pt[:1, :w])

        # logsumexp
        expv = sb.tile([128, 2, W], dt.float32, tag="oh", name="expv")
        nc.scalar.activation(expv[:, :, HALO:], logits[:, :, HALO:], AF.Exp)
        lserow = sb.tile([1, W], dt.float32, tag="lserow", name="lserow")
        for (lo, w) in _halves(HALO):
            pt = pr_tile(w)
            for cc in range(2):
                nc.tensor.matmul(pt[:1, :w], ones_cf[:], expv[:, cc, lo:lo + w],
                                 start=(cc == 0), stop=(cc == 1))
            nc.scalar.activation(lserow[:, lo:lo + w], pt[:1, :w], AF.Ln)

        nllrow = sb.tile([1, W], dt.float32, tag="nllrow", name="nllrow")
        nc.vector.tensor_tensor(nllrow[:, HALO:], lserow[:, HALO:],
                                ltrow[:, HALO:], op=OP.subtract)
        nc.sync.dma_start(out=d_nll[:], in_=nllrow[0:1, HALO:W])

    nc.compile()
    return nc


def _prep_host(inputs):
    """Host-side sharding/layout prep. Returns (shared_map, per_core_maps)."""
    bf16 = ml_dtypes.bfloat16
    inp = np.asarray(inputs["inp"])
    tgt = np.asarray(inputs["tgt"])
    emb = np.asarray(inputs["emb"], dtype=np.float32)
    w0s = np.asarray(inputs["w0s"], dtype=np.float32)
    w1s = np.asarray(inputs["w1s"], dtype=np.float32)
    w2s = np.asarray(inputs["w2s"], dtype=np.float32)
    out_w = np.asarray(inputs["out_w"], dtype=np.float32)
    out_b = np.asarray(inputs["out_b"], dtype=np.float32)

    shared = {}
    shared["emb32"] = np.ascontiguousarray(
        emb.reshape(2, 128, 1024).transpose(1, 0, 2).reshape(128, 2048))
    shared["owT"] = np.ascontiguousarray(
        out_w[:, :, 0].T.reshape(8, 128, 256).transpose(1, 0, 2)
        .reshape(128, 2048)).astype(bf16)
    shared["ob2"] = np.ascontiguousarray(out_b.reshape(2, 128).T)
    # feature embedding (match reference _feature_embd, fp32 math)
    f = np.arange(F, dtype=np.float32)[:, None] + np.float32(1.0)
    additive = f % np.float32(2.0)
    f = (f - additive) / np.float32(2.0)
    f = f * np.float32(8.0 / F) - np.float32(math.log(C / (2.0 * math.pi)))
    fe = (np.exp(f) + additive * np.float32(math.pi))[:, 0]  # [F]
    shared["fe4"] = np.ascontiguousarray(fe.reshape(4, 128).T)
    shared["triU"] = np.triu(np.ones((128, 128), dtype=np.float32)).astype(bf16)
    for l in range(L):
        for j in range(3):
            shared[f"w0_{l}{j}"] = np.ascontiguousarray(
                w0s[l, j, :, :, 0].T.reshape(4, 128, 1024).transpose(1, 0, 2)
                .reshape(128, 4096)).astype(bf16)
            shared[f"w1_{l}{j}"] = np.ascontiguousarray(
                w1s[l, j].transpose(2, 1, 0).reshape(3, 8, 128, 8, 128)
                .transpose(2, 3, 0, 1, 4).reshape(128, 24576)).astype(bf16)
            shared[f"w2_{l}{j}"] = np.ascontiguousarray(
                w2s[l, j, :, :, 0].T.reshape(8, 128, 512).transpose(1, 0, 2)
                .reshape(128, 4096)).astype(bf16)

    per_core = []
    for core in range(NCORES):
        b, q = core // 4, core % 4
        pos0 = q * CHUNK
        absidx = pos0 - HALO + np.arange(W)
        valid = absidx >= 0
        inprow = np.where(valid, inp[b, np.where(valid, absidx, 0)], PAD_IDX
                          ).astype(np.float32)
        idvrow = np.where(valid, 1.0 / (absidx + 1.0), 1.0).astype(np.float32)
        m4row = valid[:HALO].astype(np.float32)
        tgtrow = tgt[b, pos0:pos0 + CHUNK].astype(np.float32)
        m = dict(shared)
        m["inprow"] = inprow
        m["tgtrow"] = tgtrow
        m["idvrow"] = idvrow
        m["m4row"] = m4row
        per_core.append(m)
    return per_core


def kernel(**inputs):
    if "nc" not in _CACHE:
        _CACHE["nc"] = _build()
    nc = _CACHE["nc"]
    in_maps = _prep_host(inputs)
    trace = TRACE
    if trace:
        try:
            from antenv.axon_hooks import get_axon_ntff_profile_hook  # noqa: F401
        except ImportError:
            trace = False
    res = bass_utils.run_bass_kernel_spmd(nc, in_maps, core_ids=list(range(NCORES)),
                                          trace=trace)
    if trace and res.exec_time_ns is not None:
        _CACHE["exec_time_ns"] = res.exec_time_ns
    nll = np.concatenate([r["nll"][0] for r in res.results])
    return np.float32(nll.mean())
